# revision 15
# baseline (speedup 1.0000x reference)
"""Multi-head causal self-attention (32 heads, RoPE) on 8 Trainium2 cores.

Tensor-parallel over heads: core c owns heads 4c..4c+3 (512 of 4096 qkv dims).
Each core computes q/k/v projections for its heads, RoPE, causal softmax
attention, and a partial o-projection; the host sums the 8 partials.

Layouts (per core):
  xT    [4096 hs, 4096 rows]  bf16   rows = b*2048 + t
  qT/kT [512 d, 4096 rows]    bf16   (transposed: head dim on partitions)
  v     [4096 rows, 512 d]    bf16   (row-major)
  oT    [512 d, 4096 rows]    bf16   normalized attention output
  out   [4096 cols, 4096 rows] f32   partial of (attn_out @ wo)^T

Softmax runs on transposed scores sT[j,i] (keys on partitions): no-max-sub
exp (scores ~N(0,1)), column sums via ones-matmul on the PE, late
normalization with a partition-broadcast reciprocal.

RoPE trig tables and the causal mask are baked into the NEFF as Const
tensors; the executable takes only xT, the packed qkv weights, and wo.
The jitted program is AOT-compiled with bass_effect suppressed
(fast_dispatch_compile) so steady-state dispatch takes the C++ fast path.
"""
import sys

for _p in ("/opt/trn_rl_repo", "/root/.axon_site/_ro/trn_rl_repo"):
    if _p not in sys.path:
        sys.path.append(_p)

import numpy as np
import ml_dtypes

import concourse.bacc as bacc
import concourse.mybir as mybir
import concourse.tile as tile

BF16 = mybir.dt.bfloat16
F32 = mybir.dt.float32
BFNP = ml_dtypes.bfloat16

N_CORES = 8
BS, SL, HS = 2, 2048, 4096
NH, HD = 32, 128
HPC = NH // N_CORES          # heads per core = 4
DPC = HPC * HD               # qkv dims per core = 512
ROWS = BS * SL               # 4096
P = 128
MC = 512                     # m-chunk (rows) width
NMC = ROWS // MC             # 8 m-chunks
NKT = HS // P                # 32 contraction tiles
NIC = SL // MC               # 4 query chunks per sequence
NJT = SL // P                # 16 key tiles per sequence
SCALE = float(HD) ** -0.5
ROPE_THETA = 10000.0

ExpF = mybir.ActivationFunctionType.Exp
CopyF = mybir.ActivationFunctionType.Copy


def _trig_tables():
    """RoPE cos/sin in the kernel's transposed layout, with the q-side
    pre-scaled by 1/sqrt(hd) and the rotate-half sign folded into sin."""
    inv_freq = 1.0 / (ROPE_THETA ** (np.arange(0, HD, 2, dtype=np.float32) / HD))
    pos = np.arange(SL, dtype=np.float32)
    freqs = pos[:, None] * inv_freq[None, :]
    emb = np.concatenate([freqs, freqs], axis=1)          # [SL, HD]
    cosT = np.cos(emb).astype(np.float32).T               # [HD, SL]
    sinT = np.sin(emb).astype(np.float32).T
    sign = np.ones((HD, 1), np.float32)
    sign[:HD // 2] = -1.0
    cosq = np.ascontiguousarray(np.tile(cosT, (1, BS)) * SCALE)
    sinq = np.ascontiguousarray(np.tile(sinT, (1, BS)) * sign * SCALE)
    cosk = np.ascontiguousarray(np.tile(cosT, (1, BS)))
    sink = np.ascontiguousarray(np.tile(sinT, (1, BS)) * sign)
    return cosq, sinq, cosk, sink


def _mask_table():
    jj = np.arange(P)[:, None]
    ii = np.arange(MC)[None, :]
    return np.concatenate(
        [(t * P + jj <= ii) for t in range(4)], axis=1).astype(BFNP)


def build_program():
    nc = bacc.Bacc("TRN2", target_bir_lowering=False, debug=False,
                   num_devices=N_CORES)

    xT_d = nc.dram_tensor("xT", [HS, ROWS], BF16, kind="ExternalInput").ap()
    wqkv_d = nc.dram_tensor("wqkv", [HS, 3 * DPC], BF16,
                            kind="ExternalInput").ap()
    wo_d = nc.dram_tensor("wo", [DPC, HS], BF16, kind="ExternalInput").ap()
    out_d = nc.dram_tensor("out", [HS, ROWS], BF16, kind="ExternalOutput").ap()

    cosq_np, sinq_np, cosk_np, sink_np = _trig_tables()
    cosq_d = nc.inline_tensor(cosq_np, name="cosq").ap()
    sinq_d = nc.inline_tensor(sinq_np, name="sinq").ap()
    cosk_d = nc.inline_tensor(cosk_np, name="cosk").ap()
    sink_d = nc.inline_tensor(sink_np, name="sink").ap()
    mask_np = np.asarray(_mask_table(), dtype=np.float32)
    mask_d = nc.inline_tensor(mask_np.astype(BFNP), name="mask").ap()

    qT_d = nc.dram_tensor("qT_i", [DPC, ROWS], BF16).ap()
    oT_d = nc.dram_tensor("oT_i", [DPC, ROWS], BF16).ap()
    kT_d = nc.dram_tensor("kT_i", [DPC, ROWS], BF16).ap()
    v_d = nc.dram_tensor("v_i", [ROWS, DPC], BF16).ap()

    with tile.TileContext(nc) as tc:
        with tc.tile_pool(name="const", bufs=1) as const_pool:
            ones_sb = const_pool.tile([P, P], BF16, tag="ones")
            nc.vector.memset(ones_sb[:], 1.0)

            # ---------------- Phase 1: q/k/v projections + RoPE ----------
            with (
                tc.tile_pool(name="wqk", bufs=1) as wqk_pool,
                tc.tile_pool(name="xb", bufs=2) as x_pool,
                tc.tile_pool(name="wvt", bufs=3) as wv_pool,
                tc.tile_pool(name="trig", bufs=2) as trig_pool,
                tc.tile_pool(name="rope", bufs=3) as rope_pool,
                tc.tile_pool(name="qko", bufs=4) as qko_pool,
                tc.tile_pool(name="vo", bufs=3) as vo_pool,
                tc.tile_pool(name="psv", bufs=1, space="PSUM") as ps_v,
                tc.tile_pool(name="psqk", bufs=2, space="PSUM") as ps_qk,
            ):
                wq_sb = wqk_pool.tile([P, NKT * DPC], BF16, tag="wq")
                wk_sb = wqk_pool.tile([P, NKT * DPC], BF16, tag="wk")

                xtiles = {}

                def load_x(mc):
                    ms = mc * MC
                    xblk = x_pool.tile([P, NKT * MC], BF16, tag="xblk",
                                       name=f"xblk{mc}")
                    if mc == 0:
                        # land the k=0 slice first so the opening matmul
                        # doesn't wait on the full 2 MB block
                        nc.sync.dma_start(
                            xblk[:, 0:MC], xT_d[0:P, ms:ms + MC])
                        nc.sync.dma_start(
                            xblk[:, MC:].rearrange("p (k m) -> p k m",
                                                   k=NKT - 1),
                            xT_d[P:, ms:ms + MC].rearrange(
                                "(k p) m -> p k m", p=P),
                        )
                    else:
                        nc.sync.dma_start(
                            xblk[:].rearrange("p (k m) -> p k m", k=NKT),
                            xT_d[:, ms:ms + MC].rearrange(
                                "(k p) m -> p k m", p=P),
                        )
                    xtiles[mc] = xblk

                load_x(0)
                for mc in range(NMC):
                    ms = mc * MC
                    if mc + 1 < NMC:
                        # prefetch the next row chunk ahead of this chunk's
                        # stores so the sync stream never head-of-line
                        # blocks the load
                        load_x(mc + 1)
                    xblk = xtiles.pop(mc)
                    # --- v = x @ wv, row-major [rows, 512] ---
                    psv_t = [ps_v.tile([P, DPC], F32, tag=f"v{jj}",
                                       name=f"psv{jj}")
                             for jj in range(MC // P)]
                    for k in range(NKT):
                        wvt = wv_pool.tile([P, DPC], BF16)
                        nc.sync.dma_start(
                            wvt[:], wqkv_d[k * P:(k + 1) * P, 2 * DPC:3 * DPC])
                        for jj in range(MC // P):
                            nc.tensor.matmul(
                                psv_t[jj][:],
                                xblk[:, k * MC + jj * P: k * MC + (jj + 1) * P],
                                wvt[:],
                                start=(k == 0), stop=(k == NKT - 1),
                            )
                    if mc == 0:
                        # issued after the first v-projection work so the
                        # 8 MB q/k weight loads don't delay the first matmul
                        nc.sync.dma_start(
                            wq_sb[:].rearrange("p (k n) -> p k n", k=NKT),
                            wqkv_d[:, 0:DPC]
                                .rearrange("(k p) n -> p k n", p=P),
                        )
                        nc.sync.dma_start(
                            wk_sb[:].rearrange("p (k n) -> p k n", k=NKT),
                            wqkv_d[:, DPC:2 * DPC]
                                .rearrange("(k p) n -> p k n", p=P),
                        )
                    for jj in range(MC // P):
                        vout = vo_pool.tile([P, DPC], BF16)
                        nc.vector.tensor_copy(vout[:], psv_t[jj][:])
                        r0 = ms + jj * P
                        nc.sync.dma_start(v_d[r0:r0 + P, :], vout[:])

                    # --- qT / kT with fused RoPE ---
                    cq = trig_pool.tile([P, MC], F32, tag="cq")
                    sq = trig_pool.tile([P, MC], F32, tag="sq")
                    ck = trig_pool.tile([P, MC], F32, tag="ck")
                    sk = trig_pool.tile([P, MC], F32, tag="sk")
                    nc.sync.dma_start(cq[:], cosq_d[:, ms:ms + MC])
                    nc.sync.dma_start(sq[:], sinq_d[:, ms:ms + MC])
                    nc.sync.dma_start(ck[:], cosk_d[:, ms:ms + MC])
                    nc.sync.dma_start(sk[:], sink_d[:, ms:ms + MC])

                    for w_sb, cos_t, sin_t, dest in (
                        (wq_sb, cq, sq, qT_d),
                        (wk_sb, ck, sk, kT_d),
                    ):
                        for nt in range(DPC // P):
                            psq = ps_qk.tile([P, MC], F32)
                            for k in range(NKT):
                                nc.tensor.matmul(
                                    psq[:],
                                    w_sb[:, k * DPC + nt * P: k * DPC + (nt + 1) * P],
                                    xblk[:, k * MC:(k + 1) * MC],
                                    start=(k == 0), stop=(k == NKT - 1),
                                )
                            cp = rope_pool.tile([P, MC], F32, tag="cp")
                            nc.scalar.activation(cp[:], psq[:], CopyF)
                            rot = rope_pool.tile([P, MC], F32, tag="rot")
                            nc.sync.dma_start(rot[0:64, :], cp[64:128, :])
                            nc.sync.dma_start(rot[64:128, :], cp[0:64, :])
                            tmp = rope_pool.tile([P, MC], F32, tag="tmp")
                            nc.vector.tensor_mul(tmp[:], psq[:], cos_t[:])
                            nc.vector.tensor_mul(rot[:], rot[:], sin_t[:])
                            ob = qko_pool.tile([P, MC], BF16)
                            nc.vector.tensor_add(ob[:], tmp[:], rot[:])
                            nc.sync.dma_start(
                                dest[nt * P:(nt + 1) * P, ms:ms + MC], ob[:])

            # ---------------- Phase 2: causal attention ------------------
            # wo_pool wraps phases 2+3 so the 8 MB o-projection weight load
            # can stream during attention instead of stalling phase 3.
            with (
                tc.tile_pool(name="wo3", bufs=1) as wo_pool,
                tc.tile_pool(name="mask2", bufs=1) as mask_pool,
                tc.tile_pool(name="ost", bufs=3) as ost_pool,
                tc.tile_pool(name="qk2", bufs=2) as qk2_pool,
                tc.tile_pool(name="v2", bufs=2) as v2_pool,
                tc.tile_pool(name="expb", bufs=6) as exp_pool,
                tc.tile_pool(name="norm", bufs=3) as norm_pool,
                tc.tile_pool(name="pss", bufs=2, space="PSUM") as ps_s,
                tc.tile_pool(name="pso", bufs=2, space="PSUM") as ps_o,
                tc.tile_pool(name="psc", bufs=2, space="PSUM") as ps_c,
                tc.tile_pool(name="ot3", bufs=2) as ot3_pool,
                tc.tile_pool(name="ev", bufs=4) as ev_pool,
                tc.tile_pool(name="psp", bufs=2, space="PSUM") as ps_p,
            ):
                wo_sb = wo_pool.tile([P, HPC * HS], BF16, tag="wo")
                mask_sb = mask_pool.tile([P, 4 * MC], BF16, tag="mask")
                nc.sync.dma_start(mask_sb[:], mask_d[:])

                pairs = [(h, b) for h in range(HPC) for b in range(BS)]
                ptiles = {}

                def load_pair(idx):
                    h, b = pairs[idx]
                    c0 = b * SL
                    qt = qk2_pool.tile([P, SL], BF16, tag="q",
                                       name=f"q{idx}")
                    kt = qk2_pool.tile([P, SL], BF16, tag="k",
                                       name=f"k{idx}")
                    nc.sync.dma_start(
                        qt[:], qT_d[h * P:(h + 1) * P, c0:c0 + SL])
                    nc.sync.dma_start(
                        kt[:], kT_d[h * P:(h + 1) * P, c0:c0 + SL])
                    vt = v2_pool.tile([P, NJT * HD], BF16, tag="vt",
                                      name=f"vt{idx}")
                    nc.sync.dma_start(
                        vt[:].rearrange("p (j d) -> p j d", j=NJT),
                        v_d[c0:c0 + SL, h * HD:(h + 1) * HD]
                            .rearrange("(j p) d -> p j d", p=P),
                    )
                    ptiles[idx] = (qt, kt, vt)

                load_pair(0)
                for idx in range(len(pairs)):
                    if idx + 1 < len(pairs):
                        load_pair(idx + 1)
                    if idx == 0:
                        # behind the first two pairs' tiles, ahead of
                        # everything phase 3 needs
                        nc.sync.dma_start(
                            wo_sb[:].rearrange("p (a c) -> p a c", a=HPC),
                            wo_d.rearrange("(a p) c -> p a c", p=P),
                        )
                    h, b = pairs[idx]
                    c0 = b * SL
                    qt, kt, vt = ptiles.pop(idx)
                    if True:
                        for ic in range(NIC):
                            njt = 4 * (ic + 1)
                            ps_out = ps_o.tile([P, MC], F32)
                            ps_sum = ps_c.tile([P, MC], F32)
                            for jt in range(njt):
                                ps_sc = ps_s.tile([P, MC], F32)
                                nc.tensor.matmul(
                                    ps_sc[:],
                                    kt[:, jt * P:(jt + 1) * P],
                                    qt[:, ic * MC:(ic + 1) * MC],
                                    start=True, stop=True,
                                )
                                et = exp_pool.tile([P, MC], BF16)
                                nc.scalar.activation(et[:], ps_sc[:], ExpF)
                                if jt >= 4 * ic:
                                    t = jt - 4 * ic
                                    nc.vector.tensor_mul(
                                        et[:], et[:],
                                        mask_sb[:, t * MC:(t + 1) * MC])
                                nc.tensor.matmul(
                                    ps_out[:],
                                    vt[:, jt * HD:(jt + 1) * HD],
                                    et[:],
                                    start=(jt == 0), stop=(jt == njt - 1),
                                )
                                nc.tensor.matmul(
                                    ps_sum[:],
                                    ones_sb[:],
                                    et[:],
                                    start=(jt == 0), stop=(jt == njt - 1),
                                )
                            bcast = norm_pool.tile([P, MC], F32, tag="bcast")
                            nc.vector.reciprocal(bcast[:], ps_sum[:])
                            ost = ost_pool.tile([P, MC], BF16)
                            nc.vector.tensor_mul(
                                ost[:], ps_out[:], bcast[:])
                            nc.sync.dma_start(
                                oT_d[h * P:(h + 1) * P,
                                     c0 + ic * MC:c0 + (ic + 1) * MC],
                                ost[:])

                # ---------------- Phase 3: partial o-projection ----------
                # same pool scope as phase 2: wo_sb is already streaming in,
                # and ot loads are prefetched one row-chunk ahead so they
                # never queue behind this chunk's output stores.
                otiles = {}

                def load_ot(ic2):
                    ots = [ot3_pool.tile([P, MC], BF16, tag=f"ot{h}",
                                         name=f"ot{h}_{ic2}")
                           for h in range(HPC)]
                    for h in range(HPC):
                        nc.sync.dma_start(
                            ots[h][:],
                            oT_d[h * P:(h + 1) * P,
                                 ic2 * MC:(ic2 + 1) * MC])
                    otiles[ic2] = ots

                load_ot(0)
                for ic2 in range(ROWS // MC):
                    if ic2 + 1 < ROWS // MC:
                        load_ot(ic2 + 1)
                    ot_ic = otiles.pop(ic2)
                    for ct in range(HS // P):
                        psp = ps_p.tile([P, MC], F32)
                        for h in range(HPC):
                            nc.tensor.matmul(
                                psp[:],
                                wo_sb[:, h * HS + ct * P: h * HS + (ct + 1) * P],
                                ot_ic[h][:],
                                start=(h == 0), stop=(h == HPC - 1),
                            )
                        ev = ev_pool.tile([P, MC], BF16)
                        nc.any.tensor_copy(ev[:], psp[:])
                        nc.sync.dma_start(
                            out_d[ct * P:(ct + 1) * P,
                                  ic2 * MC:(ic2 + 1) * MC], ev[:])

    nc.compile()
    return nc


def _host_inputs(hidden_states, wq, wk, wv, wo):
    """Per-core input dicts: xT (replicated), packed wqkv slice, wo slice."""
    x = np.asarray(hidden_states, dtype=np.float32).reshape(ROWS, HS)
    xT = np.ascontiguousarray(x.T).astype(BFNP)

    wq = np.asarray(wq, np.float32)
    wk = np.asarray(wk, np.float32)
    wv = np.asarray(wv, np.float32)
    wo = np.asarray(wo, np.float32)

    in_maps = []
    for c in range(N_CORES):
        s = slice(c * DPC, (c + 1) * DPC)
        wqkv = np.concatenate([wq[:, s], wk[:, s], wv[:, s]], axis=1)
        in_maps.append({
            "xT": xT,
            "wqkv": np.ascontiguousarray(wqkv).astype(BFNP),
            "wo": np.ascontiguousarray(wo[s, :]).astype(BFNP),
        })
    return in_maps


class Runner:
    """Compile the program once into a sharded PJRT executable with the
    bass effect suppressed (C++ fast-path dispatch). Inputs must be
    device-resident with the mesh sharding; use stage() for that."""

    def __init__(self, nc):
        import jax
        import concourse.mybir as _mybir
        from concourse import bass2jax
        from jax.experimental.shard_map import shard_map
        from jax.sharding import Mesh, PartitionSpec, NamedSharding

        bass2jax.install_neuronx_cc_hook()
        self.jax = jax
        partition_name = (
            nc.partition_id_tensor.name if nc.partition_id_tensor else None)
        in_names, in_avals, out_names, out_avals = [], [], [], []
        for alloc in nc.m.functions[0].allocations:
            if not isinstance(alloc, _mybir.MemoryLocationSet):
                continue
            name = alloc.memorylocations[0].name
            if alloc.kind == "ExternalInput":
                if name != partition_name:
                    in_names.append(name)
                    in_avals.append((tuple(alloc.tensor_shape),
                                     _mybir.dt.np(alloc.dtype)))
            elif alloc.kind == "ExternalOutput":
                shape = tuple(alloc.tensor_shape)
                dtype = _mybir.dt.np(alloc.dtype)
                out_names.append(name)
                out_avals.append(jax.core.ShapedArray(shape, dtype))
        self.in_names, self.out_names = in_names, out_names
        self.out_avals = out_avals
        all_names = list(in_names)
        if partition_name is not None:
            all_names = all_names + [partition_name]

        def _body(*args):
            operands = list(args)
            if partition_name is not None:
                operands.append(bass2jax.partition_id_tensor())
            outs = bass2jax._bass_exec_p.bind(
                *operands,
                out_avals=tuple(out_avals),
                in_names=tuple(all_names),
                out_names=tuple(out_names),
                lowering_input_output_aliases=(),
                sim_require_finite=True,
                sim_require_nnan=True,
                nc=nc,
            )
            return tuple(outs)

        devices = jax.devices()[:N_CORES]
        self.mesh = Mesh(np.asarray(devices), ("core",))
        self.sharding = NamedSharding(self.mesh, PartitionSpec("core"))
        wrapped = shard_map(
            _body, mesh=self.mesh,
            in_specs=(PartitionSpec("core"),) * len(in_names),
            out_specs=(PartitionSpec("core"),) * len(out_names),
            check_rep=False,
        )
        abstract = [
            jax.ShapeDtypeStruct((N_CORES * shape[0],) + shape[1:], dtype,
                                 sharding=self.sharding)
            for shape, dtype in in_avals
        ]
        self.fn = bass2jax.fast_dispatch_compile(
            lambda: jax.jit(wrapped, keep_unused=True)
            .lower(*abstract).compile())

    def concat_inputs(self, in_maps):
        return [
            np.concatenate([np.asarray(m[name]) for m in in_maps], axis=0)
            for name in self.in_names
        ]

    def stage(self, in_maps):
        """Concatenate per-core inputs and place them on the mesh."""
        args = self.concat_inputs(in_maps)
        dev_args = [self.jax.device_put(a, self.sharding) for a in args]
        self.jax.block_until_ready(dev_args)
        return dev_args

    def run(self, in_maps):
        out_arrs = self.fn(*self.stage(in_maps))
        return [
            {
                name: np.asarray(out_arrs[i]).reshape(
                    N_CORES, *self.out_avals[i].shape)[c]
                for i, name in enumerate(self.out_names)
            }
            for c in range(N_CORES)
        ]


_RUNNER = None


def get_runner():
    global _RUNNER
    if _RUNNER is None:
        _RUNNER = Runner(build_program())
    return _RUNNER


def kernel(hidden_states, wq, wk, wv, wo):
    runner = get_runner()
    in_maps = _host_inputs(hidden_states, wq, wk, wv, wo)
    results = runner.run(in_maps)
    total = results[0]["out"].astype(np.float64)
    for c in range(1, N_CORES):
        total += results[c]["out"].astype(np.float64)
    return np.ascontiguousarray(
        total.T.reshape(BS, SL, HS)).astype(np.float32)


# revision 19
# speedup vs baseline: 1.0225x; 1.0225x over previous
"""Multi-head causal self-attention (32 heads, RoPE) on 8 Trainium2 cores.

Tensor-parallel over heads: core c owns heads 4c..4c+3 (512 of 4096 qkv dims).
Each core computes q/k/v projections for its heads, RoPE, causal softmax
attention, and a partial o-projection; the host sums the 8 partials.

Layouts (per core):
  xT    [4096 hs, 4096 rows]  bf16   rows = b*2048 + t
  qT/kT [512 d, 4096 rows]    bf16   (transposed: head dim on partitions)
  v     [4096 rows, 512 d]    bf16   (row-major)
  oT    [512 d, 4096 rows]    bf16   normalized attention output
  out   [4096 cols, 4096 rows] f32   partial of (attn_out @ wo)^T

Softmax runs on transposed scores sT[j,i] (keys on partitions): no-max-sub
exp (scores ~N(0,1)), column sums via ones-matmul on the PE, late
normalization with a partition-broadcast reciprocal.

RoPE trig tables and the causal mask are baked into the NEFF as Const
tensors; the executable takes only xT, the packed qkv weights, and wo.
The jitted program is AOT-compiled with bass_effect suppressed
(fast_dispatch_compile) so steady-state dispatch takes the C++ fast path.
"""
import sys

for _p in ("/opt/trn_rl_repo", "/root/.axon_site/_ro/trn_rl_repo"):
    if _p not in sys.path:
        sys.path.append(_p)

import numpy as np
import ml_dtypes

import concourse.bacc as bacc
import concourse.mybir as mybir
import concourse.tile as tile

BF16 = mybir.dt.bfloat16
F32 = mybir.dt.float32
BFNP = ml_dtypes.bfloat16

N_CORES = 8
BS, SL, HS = 2, 2048, 4096
NH, HD = 32, 128
HPC = NH // N_CORES          # heads per core = 4
DPC = HPC * HD               # qkv dims per core = 512
ROWS = BS * SL               # 4096
P = 128
MC = 512                     # m-chunk (rows) width
NMC = ROWS // MC             # 8 m-chunks
NKT = HS // P                # 32 contraction tiles
NIC = SL // MC               # 4 query chunks per sequence
NJT = SL // P                # 16 key tiles per sequence
SCALE = float(HD) ** -0.5
ROPE_THETA = 10000.0

ExpF = mybir.ActivationFunctionType.Exp
CopyF = mybir.ActivationFunctionType.Copy


def _trig_tables():
    """RoPE cos/sin in the kernel's transposed layout, with the q-side
    pre-scaled by 1/sqrt(hd) and the rotate-half sign folded into sin."""
    inv_freq = 1.0 / (ROPE_THETA ** (np.arange(0, HD, 2, dtype=np.float32) / HD))
    pos = np.arange(SL, dtype=np.float32)
    freqs = pos[:, None] * inv_freq[None, :]
    emb = np.concatenate([freqs, freqs], axis=1)          # [SL, HD]
    cosT = np.cos(emb).astype(np.float32).T               # [HD, SL]
    sinT = np.sin(emb).astype(np.float32).T
    sign = np.ones((HD, 1), np.float32)
    sign[:HD // 2] = -1.0
    cosq = np.ascontiguousarray(np.tile(cosT, (1, BS)) * SCALE)
    sinq = np.ascontiguousarray(np.tile(sinT, (1, BS)) * sign * SCALE)
    cosk = np.ascontiguousarray(np.tile(cosT, (1, BS)))
    sink = np.ascontiguousarray(np.tile(sinT, (1, BS)) * sign)
    return cosq, sinq, cosk, sink


def _mask_table():
    jj = np.arange(P)[:, None]
    ii = np.arange(MC)[None, :]
    return np.concatenate(
        [(t * P + jj <= ii) for t in range(4)], axis=1).astype(BFNP)


def build_program():
    nc = bacc.Bacc("TRN2", target_bir_lowering=False, debug=False,
                   num_devices=N_CORES)

    xT_d = nc.dram_tensor("xT", [HS, ROWS], BF16, kind="ExternalInput").ap()
    wqkv_d = nc.dram_tensor("wqkv", [HS, 3 * DPC], BF16,
                            kind="ExternalInput").ap()
    wo_d = nc.dram_tensor("wo", [DPC, HS], BF16, kind="ExternalInput").ap()
    out_d = nc.dram_tensor("out", [HS, ROWS], BF16, kind="ExternalOutput").ap()

    cosq_np, sinq_np, cosk_np, sink_np = _trig_tables()
    cosq_d = nc.inline_tensor(cosq_np, name="cosq").ap()
    sinq_d = nc.inline_tensor(sinq_np, name="sinq").ap()
    cosk_d = nc.inline_tensor(cosk_np, name="cosk").ap()
    sink_d = nc.inline_tensor(sink_np, name="sink").ap()
    mask_np = np.asarray(_mask_table(), dtype=np.float32)
    mask_d = nc.inline_tensor(mask_np.astype(BFNP), name="mask").ap()

    qT_d = nc.dram_tensor("qT_i", [DPC, ROWS], BF16).ap()
    oT_d = nc.dram_tensor("oT_i", [DPC, ROWS], BF16).ap()
    kT_d = nc.dram_tensor("kT_i", [DPC, ROWS], BF16).ap()
    v_d = nc.dram_tensor("v_i", [ROWS, DPC], BF16).ap()

    with tile.TileContext(nc) as tc:
        with tc.tile_pool(name="const", bufs=1) as const_pool:
            ones_sb = const_pool.tile([P, P], BF16, tag="ones")
            nc.vector.memset(ones_sb[:], 1.0)

            # ---------------- Phase 1: q/k/v projections + RoPE ----------
            with (
                tc.tile_pool(name="wqk", bufs=1) as wqk_pool,
                tc.tile_pool(name="xb", bufs=2) as x_pool,
                tc.tile_pool(name="wvt", bufs=3) as wv_pool,
                tc.tile_pool(name="trig", bufs=2) as trig_pool,
                tc.tile_pool(name="rope", bufs=3) as rope_pool,
                tc.tile_pool(name="qko", bufs=4) as qko_pool,
                tc.tile_pool(name="vo", bufs=3) as vo_pool,
                tc.tile_pool(name="psv", bufs=1, space="PSUM") as ps_v,
                tc.tile_pool(name="psqk", bufs=2, space="PSUM") as ps_qk,
            ):
                wq_sb = wqk_pool.tile([P, NKT * DPC], BF16, tag="wq")
                wk_sb = wqk_pool.tile([P, NKT * DPC], BF16, tag="wk")

                xtiles = {}

                def load_x(mc):
                    ms = mc * MC
                    xblk = x_pool.tile([P, NKT * MC], BF16, tag="xblk",
                                       name=f"xblk{mc}")
                    if mc == 0:
                        # land the k=0 slice first so the opening matmul
                        # doesn't wait on the full 2 MB block
                        nc.sync.dma_start(
                            xblk[:, 0:MC], xT_d[0:P, ms:ms + MC])
                        nc.sync.dma_start(
                            xblk[:, MC:].rearrange("p (k m) -> p k m",
                                                   k=NKT - 1),
                            xT_d[P:, ms:ms + MC].rearrange(
                                "(k p) m -> p k m", p=P),
                        )
                    else:
                        nc.sync.dma_start(
                            xblk[:].rearrange("p (k m) -> p k m", k=NKT),
                            xT_d[:, ms:ms + MC].rearrange(
                                "(k p) m -> p k m", p=P),
                        )
                    xtiles[mc] = xblk

                load_x(0)
                for mc in range(NMC):
                    ms = mc * MC
                    xblk = xtiles.pop(mc)
                    # --- v = x @ wv, row-major [rows, 512] ---
                    psv_t = [ps_v.tile([P, DPC], F32, tag=f"v{jj}",
                                       name=f"psv{jj}")
                             for jj in range(MC // P)]
                    for k in range(NKT):
                        wvt = wv_pool.tile([P, DPC], BF16)
                        nc.sync.dma_start(
                            wvt[:], wqkv_d[k * P:(k + 1) * P, 2 * DPC:3 * DPC])
                        for jj in range(MC // P):
                            nc.tensor.matmul(
                                psv_t[jj][:],
                                xblk[:, k * MC + jj * P: k * MC + (jj + 1) * P],
                                wvt[:],
                                start=(k == 0), stop=(k == NKT - 1),
                            )
                    if mc == 0:
                        # issued after the first v-projection work so the
                        # 8 MB q/k weight loads don't delay the first matmul
                        nc.sync.dma_start(
                            wq_sb[:].rearrange("p (k n) -> p k n", k=NKT),
                            wqkv_d[:, 0:DPC]
                                .rearrange("(k p) n -> p k n", p=P),
                        )
                        nc.sync.dma_start(
                            wk_sb[:].rearrange("p (k n) -> p k n", k=NKT),
                            wqkv_d[:, DPC:2 * DPC]
                                .rearrange("(k p) n -> p k n", p=P),
                        )
                    if mc + 1 < NMC:
                        # prefetch the next row chunk after this chunk's
                        # v-section DMAs so it doesn't delay the wv stream,
                        # but ahead of the stores below so the sync stream
                        # never head-of-line blocks the load
                        load_x(mc + 1)
                    for jj in range(MC // P):
                        vout = vo_pool.tile([P, DPC], BF16)
                        nc.vector.tensor_copy(vout[:], psv_t[jj][:])
                        r0 = ms + jj * P
                        nc.sync.dma_start(v_d[r0:r0 + P, :], vout[:])

                    # --- qT / kT with fused RoPE ---
                    cq = trig_pool.tile([P, MC], F32, tag="cq")
                    sq = trig_pool.tile([P, MC], F32, tag="sq")
                    ck = trig_pool.tile([P, MC], F32, tag="ck")
                    sk = trig_pool.tile([P, MC], F32, tag="sk")
                    nc.sync.dma_start(cq[:], cosq_d[:, ms:ms + MC])
                    nc.sync.dma_start(sq[:], sinq_d[:, ms:ms + MC])
                    nc.sync.dma_start(ck[:], cosk_d[:, ms:ms + MC])
                    nc.sync.dma_start(sk[:], sink_d[:, ms:ms + MC])

                    for w_sb, cos_t, sin_t, dest in (
                        (wq_sb, cq, sq, qT_d),
                        (wk_sb, ck, sk, kT_d),
                    ):
                        for nt in range(DPC // P):
                            psq = ps_qk.tile([P, MC], F32)
                            for k in range(NKT):
                                nc.tensor.matmul(
                                    psq[:],
                                    w_sb[:, k * DPC + nt * P: k * DPC + (nt + 1) * P],
                                    xblk[:, k * MC:(k + 1) * MC],
                                    start=(k == 0), stop=(k == NKT - 1),
                                )
                            cp = rope_pool.tile([P, MC], F32, tag="cp")
                            nc.scalar.activation(cp[:], psq[:], CopyF)
                            rot = rope_pool.tile([P, MC], F32, tag="rot")
                            nc.sync.dma_start(rot[0:64, :], cp[64:128, :])
                            nc.sync.dma_start(rot[64:128, :], cp[0:64, :])
                            tmp = rope_pool.tile([P, MC], F32, tag="tmp")
                            nc.vector.tensor_mul(tmp[:], psq[:], cos_t[:])
                            nc.vector.tensor_mul(rot[:], rot[:], sin_t[:])
                            ob = qko_pool.tile([P, MC], BF16)
                            nc.vector.tensor_add(ob[:], tmp[:], rot[:])
                            nc.sync.dma_start(
                                dest[nt * P:(nt + 1) * P, ms:ms + MC], ob[:])

            # ---------------- Phase 2: causal attention ------------------
            # wo_pool wraps phases 2+3 so the 8 MB o-projection weight load
            # can stream during attention instead of stalling phase 3.
            with (
                tc.tile_pool(name="wo3", bufs=1) as wo_pool,
                tc.tile_pool(name="mask2", bufs=1) as mask_pool,
                tc.tile_pool(name="ost", bufs=3) as ost_pool,
                tc.tile_pool(name="qk2", bufs=2) as qk2_pool,
                tc.tile_pool(name="v2", bufs=2) as v2_pool,
                tc.tile_pool(name="expb", bufs=6) as exp_pool,
                tc.tile_pool(name="norm", bufs=3) as norm_pool,
                tc.tile_pool(name="pss", bufs=3, space="PSUM") as ps_s,
                tc.tile_pool(name="pso", bufs=2, space="PSUM") as ps_o,
                tc.tile_pool(name="psc", bufs=2, space="PSUM") as ps_c,
                tc.tile_pool(name="ot3", bufs=2) as ot3_pool,
                tc.tile_pool(name="ev", bufs=4) as ev_pool,
            ):
                wo_sb = wo_pool.tile([P, HPC * HS], BF16, tag="wo")
                mask_sb = mask_pool.tile([P, 4 * MC], BF16, tag="mask")
                nc.sync.dma_start(mask_sb[:], mask_d[:])

                pairs = [(h, b) for h in range(HPC) for b in range(BS)]
                ptiles = {}

                def load_pair(idx):
                    h, b = pairs[idx]
                    c0 = b * SL
                    qt = qk2_pool.tile([P, SL], BF16, tag="q",
                                       name=f"q{idx}")
                    kt = qk2_pool.tile([P, SL], BF16, tag="k",
                                       name=f"k{idx}")
                    nc.sync.dma_start(
                        qt[:], qT_d[h * P:(h + 1) * P, c0:c0 + SL])
                    nc.sync.dma_start(
                        kt[:], kT_d[h * P:(h + 1) * P, c0:c0 + SL])
                    vt = v2_pool.tile([P, NJT * HD], BF16, tag="vt",
                                      name=f"vt{idx}")
                    nc.sync.dma_start(
                        vt[:].rearrange("p (j d) -> p j d", j=NJT),
                        v_d[c0:c0 + SL, h * HD:(h + 1) * HD]
                            .rearrange("(j p) d -> p j d", p=P),
                    )
                    ptiles[idx] = (qt, kt, vt)

                load_pair(0)
                for idx in range(len(pairs)):
                    if idx + 1 < len(pairs):
                        load_pair(idx + 1)
                    if idx == 0:
                        # behind the first two pairs' tiles, ahead of
                        # everything phase 3 needs
                        nc.sync.dma_start(
                            wo_sb[:].rearrange("p (a c) -> p a c", a=HPC),
                            wo_d.rearrange("(a p) c -> p a c", p=P),
                        )
                    h, b = pairs[idx]
                    c0 = b * SL
                    qt, kt, vt = ptiles.pop(idx)
                    if True:
                        for ic in range(NIC):
                            njt = 4 * (ic + 1)
                            ps_out = ps_o.tile([P, MC], F32)
                            ps_sum = ps_c.tile([P, MC], F32)
                            for jt in range(njt):
                                ps_sc = ps_s.tile([P, MC], F32)
                                nc.tensor.matmul(
                                    ps_sc[:],
                                    kt[:, jt * P:(jt + 1) * P],
                                    qt[:, ic * MC:(ic + 1) * MC],
                                    start=True, stop=True,
                                )
                                et = exp_pool.tile([P, MC], BF16)
                                nc.scalar.activation(et[:], ps_sc[:], ExpF)
                                if jt >= 4 * ic:
                                    t = jt - 4 * ic
                                    nc.vector.tensor_mul(
                                        et[:], et[:],
                                        mask_sb[:, t * MC:(t + 1) * MC])
                                nc.tensor.matmul(
                                    ps_out[:],
                                    vt[:, jt * HD:(jt + 1) * HD],
                                    et[:],
                                    start=(jt == 0), stop=(jt == njt - 1),
                                )
                                nc.tensor.matmul(
                                    ps_sum[:],
                                    ones_sb[:],
                                    et[:],
                                    start=(jt == 0), stop=(jt == njt - 1),
                                )
                            bcast = norm_pool.tile([P, MC], F32, tag="bcast")
                            nc.vector.reciprocal(bcast[:], ps_sum[:])
                            ost = ost_pool.tile([P, MC], BF16)
                            nc.vector.tensor_mul(
                                ost[:], ps_out[:], bcast[:])
                            nc.sync.dma_start(
                                oT_d[h * P:(h + 1) * P,
                                     c0 + ic * MC:c0 + (ic + 1) * MC],
                                ost[:])

                # ---------------- Phase 3: partial o-projection ----------
                # same pool scope as phase 2: wo_sb is already streaming in,
                # and ot loads are prefetched one row-chunk ahead so they
                # never queue behind this chunk's output stores.
                otiles = {}

                def load_ot(ic2):
                    ots = [ot3_pool.tile([P, MC], BF16, tag=f"ot{h}",
                                         name=f"ot{h}_{ic2}")
                           for h in range(HPC)]
                    for h in range(HPC):
                        nc.sync.dma_start(
                            ots[h][:],
                            oT_d[h * P:(h + 1) * P,
                                 ic2 * MC:(ic2 + 1) * MC])
                    otiles[ic2] = ots

                load_ot(0)
                for ic2 in range(ROWS // MC):
                    if ic2 + 1 < ROWS // MC:
                        load_ot(ic2 + 1)
                    ot_ic = otiles.pop(ic2)
                    for ct in range(HS // P):
                        # reuse phase 2's (now idle) PSUM banks: 4-way ILP
                        # without exceeding the 8-bank budget
                        psp = (ps_o if ct % 2 == 0 else ps_c).tile(
                            [P, MC], F32,
                            tag="ps_out" if ct % 2 == 0 else "ps_sum")
                        for h in range(HPC):
                            nc.tensor.matmul(
                                psp[:],
                                wo_sb[:, h * HS + ct * P: h * HS + (ct + 1) * P],
                                ot_ic[h][:],
                                start=(h == 0), stop=(h == HPC - 1),
                            )
                        ev = ev_pool.tile([P, MC], BF16)
                        nc.any.tensor_copy(ev[:], psp[:])
                        nc.sync.dma_start(
                            out_d[ct * P:(ct + 1) * P,
                                  ic2 * MC:(ic2 + 1) * MC], ev[:])

    nc.compile()
    return nc


def _host_inputs(hidden_states, wq, wk, wv, wo):
    """Per-core input dicts: xT (replicated), packed wqkv slice, wo slice."""
    x = np.asarray(hidden_states, dtype=np.float32).reshape(ROWS, HS)
    xT = np.ascontiguousarray(x.T).astype(BFNP)

    wq = np.asarray(wq, np.float32)
    wk = np.asarray(wk, np.float32)
    wv = np.asarray(wv, np.float32)
    wo = np.asarray(wo, np.float32)

    in_maps = []
    for c in range(N_CORES):
        s = slice(c * DPC, (c + 1) * DPC)
        wqkv = np.concatenate([wq[:, s], wk[:, s], wv[:, s]], axis=1)
        in_maps.append({
            "xT": xT,
            "wqkv": np.ascontiguousarray(wqkv).astype(BFNP),
            "wo": np.ascontiguousarray(wo[s, :]).astype(BFNP),
        })
    return in_maps


class Runner:
    """Compile the program once into a sharded PJRT executable with the
    bass effect suppressed (C++ fast-path dispatch). Inputs must be
    device-resident with the mesh sharding; use stage() for that."""

    def __init__(self, nc):
        import jax
        import concourse.mybir as _mybir
        from concourse import bass2jax
        from jax.experimental.shard_map import shard_map
        from jax.sharding import Mesh, PartitionSpec, NamedSharding

        bass2jax.install_neuronx_cc_hook()
        self.jax = jax
        partition_name = (
            nc.partition_id_tensor.name if nc.partition_id_tensor else None)
        in_names, in_avals, out_names, out_avals = [], [], [], []
        for alloc in nc.m.functions[0].allocations:
            if not isinstance(alloc, _mybir.MemoryLocationSet):
                continue
            name = alloc.memorylocations[0].name
            if alloc.kind == "ExternalInput":
                if name != partition_name:
                    in_names.append(name)
                    in_avals.append((tuple(alloc.tensor_shape),
                                     _mybir.dt.np(alloc.dtype)))
            elif alloc.kind == "ExternalOutput":
                shape = tuple(alloc.tensor_shape)
                dtype = _mybir.dt.np(alloc.dtype)
                out_names.append(name)
                out_avals.append(jax.core.ShapedArray(shape, dtype))
        self.in_names, self.out_names = in_names, out_names
        self.out_avals = out_avals
        all_names = list(in_names)
        if partition_name is not None:
            all_names = all_names + [partition_name]

        def _body(*args):
            operands = list(args)
            if partition_name is not None:
                operands.append(bass2jax.partition_id_tensor())
            outs = bass2jax._bass_exec_p.bind(
                *operands,
                out_avals=tuple(out_avals),
                in_names=tuple(all_names),
                out_names=tuple(out_names),
                lowering_input_output_aliases=(),
                sim_require_finite=True,
                sim_require_nnan=True,
                nc=nc,
            )
            return tuple(outs)

        devices = jax.devices()[:N_CORES]
        self.mesh = Mesh(np.asarray(devices), ("core",))
        self.sharding = NamedSharding(self.mesh, PartitionSpec("core"))
        wrapped = shard_map(
            _body, mesh=self.mesh,
            in_specs=(PartitionSpec("core"),) * len(in_names),
            out_specs=(PartitionSpec("core"),) * len(out_names),
            check_rep=False,
        )
        abstract = [
            jax.ShapeDtypeStruct((N_CORES * shape[0],) + shape[1:], dtype,
                                 sharding=self.sharding)
            for shape, dtype in in_avals
        ]
        self.fn = bass2jax.fast_dispatch_compile(
            lambda: jax.jit(wrapped, keep_unused=True)
            .lower(*abstract).compile())

    def concat_inputs(self, in_maps):
        return [
            np.concatenate([np.asarray(m[name]) for m in in_maps], axis=0)
            for name in self.in_names
        ]

    def stage(self, in_maps):
        """Concatenate per-core inputs and place them on the mesh."""
        args = self.concat_inputs(in_maps)
        dev_args = [self.jax.device_put(a, self.sharding) for a in args]
        self.jax.block_until_ready(dev_args)
        return dev_args

    def run(self, in_maps):
        out_arrs = self.fn(*self.stage(in_maps))
        return [
            {
                name: np.asarray(out_arrs[i]).reshape(
                    N_CORES, *self.out_avals[i].shape)[c]
                for i, name in enumerate(self.out_names)
            }
            for c in range(N_CORES)
        ]


_RUNNER = None


def get_runner():
    global _RUNNER
    if _RUNNER is None:
        _RUNNER = Runner(build_program())
    return _RUNNER


def kernel(hidden_states, wq, wk, wv, wo):
    runner = get_runner()
    in_maps = _host_inputs(hidden_states, wq, wk, wv, wo)
    results = runner.run(in_maps)
    total = results[0]["out"].astype(np.float64)
    for c in range(1, N_CORES):
        total += results[c]["out"].astype(np.float64)
    return np.ascontiguousarray(
        total.T.reshape(BS, SL, HS)).astype(np.float32)


# revision 20
# speedup vs baseline: 1.0444x; 1.0214x over previous
"""Multi-head causal self-attention (32 heads, RoPE) on 8 Trainium2 cores.

Tensor-parallel over heads: core c owns heads 4c..4c+3 (512 of 4096 qkv dims).
Each core computes q/k/v projections for its heads, RoPE, causal softmax
attention, and a partial o-projection; the host sums the 8 partials.

Layouts (per core):
  xT    [4096 hs, 4096 rows]  bf16   rows = b*2048 + t
  qT/kT [512 d, 4096 rows]    bf16   (transposed: head dim on partitions)
  v     [4096 rows, 512 d]    bf16   (row-major)
  oT    [512 d, 4096 rows]    bf16   normalized attention output
  out   [4096 cols, 4096 rows] f32   partial of (attn_out @ wo)^T

Softmax runs on transposed scores sT[j,i] (keys on partitions): no-max-sub
exp (scores ~N(0,1)), column sums via ones-matmul on the PE, late
normalization with a partition-broadcast reciprocal.

RoPE trig tables and the causal mask are baked into the NEFF as Const
tensors; the executable takes only xT, the packed qkv weights, and wo.
The jitted program is AOT-compiled with bass_effect suppressed
(fast_dispatch_compile) so steady-state dispatch takes the C++ fast path.
"""
import sys

for _p in ("/opt/trn_rl_repo", "/root/.axon_site/_ro/trn_rl_repo"):
    if _p not in sys.path:
        sys.path.append(_p)

import numpy as np
import ml_dtypes

import concourse.bacc as bacc
import concourse.mybir as mybir
import concourse.tile as tile

BF16 = mybir.dt.bfloat16
F32 = mybir.dt.float32
BFNP = ml_dtypes.bfloat16

N_CORES = 8
BS, SL, HS = 2, 2048, 4096
NH, HD = 32, 128
HPC = NH // N_CORES          # heads per core = 4
DPC = HPC * HD               # qkv dims per core = 512
ROWS = BS * SL               # 4096
P = 128
MC = 512                     # m-chunk (rows) width
NMC = ROWS // MC             # 8 m-chunks
NKT = HS // P                # 32 contraction tiles
NIC = SL // MC               # 4 query chunks per sequence
NJT = SL // P                # 16 key tiles per sequence
SCALE = float(HD) ** -0.5
ROPE_THETA = 10000.0

ExpF = mybir.ActivationFunctionType.Exp
CopyF = mybir.ActivationFunctionType.Copy


def _trig_tables():
    """RoPE cos/sin in the kernel's transposed layout, with the q-side
    pre-scaled by 1/sqrt(hd) and the rotate-half sign folded into sin."""
    inv_freq = 1.0 / (ROPE_THETA ** (np.arange(0, HD, 2, dtype=np.float32) / HD))
    pos = np.arange(SL, dtype=np.float32)
    freqs = pos[:, None] * inv_freq[None, :]
    emb = np.concatenate([freqs, freqs], axis=1)          # [SL, HD]
    cosT = np.cos(emb).astype(np.float32).T               # [HD, SL]
    sinT = np.sin(emb).astype(np.float32).T
    sign = np.ones((HD, 1), np.float32)
    sign[:HD // 2] = -1.0
    cosq = np.ascontiguousarray(np.tile(cosT, (1, BS)) * SCALE)
    sinq = np.ascontiguousarray(np.tile(sinT, (1, BS)) * sign * SCALE)
    cosk = np.ascontiguousarray(np.tile(cosT, (1, BS)))
    sink = np.ascontiguousarray(np.tile(sinT, (1, BS)) * sign)
    return cosq, sinq, cosk, sink


def _mask_table():
    jj = np.arange(P)[:, None]
    ii = np.arange(MC)[None, :]
    return np.concatenate(
        [(t * P + jj <= ii) for t in range(4)], axis=1).astype(BFNP)


def build_program():
    nc = bacc.Bacc("TRN2", target_bir_lowering=False, debug=False,
                   num_devices=N_CORES)

    xT_d = nc.dram_tensor("xT", [HS, ROWS], BF16, kind="ExternalInput").ap()
    wqkv_d = nc.dram_tensor("wqkv", [HS, 3 * DPC], BF16,
                            kind="ExternalInput").ap()
    wo_d = nc.dram_tensor("wo", [DPC, HS], BF16, kind="ExternalInput").ap()
    out_d = nc.dram_tensor("out", [HS, ROWS], BF16, kind="ExternalOutput").ap()

    cosq_np, sinq_np, cosk_np, sink_np = _trig_tables()
    cosq_d = nc.inline_tensor(cosq_np, name="cosq").ap()
    sinq_d = nc.inline_tensor(sinq_np, name="sinq").ap()
    cosk_d = nc.inline_tensor(cosk_np, name="cosk").ap()
    sink_d = nc.inline_tensor(sink_np, name="sink").ap()
    mask_np = np.asarray(_mask_table(), dtype=np.float32)
    mask_d = nc.inline_tensor(mask_np.astype(BFNP), name="mask").ap()

    qT_d = nc.dram_tensor("qT_i", [DPC, ROWS], BF16).ap()
    oT_d = nc.dram_tensor("oT_i", [DPC, ROWS], BF16).ap()
    kT_d = nc.dram_tensor("kT_i", [DPC, ROWS], BF16).ap()
    v_d = nc.dram_tensor("v_i", [ROWS, DPC], BF16).ap()

    with tile.TileContext(nc) as tc:
        with tc.tile_pool(name="const", bufs=1) as const_pool:
            ones_sb = const_pool.tile([P, P], BF16, tag="ones")
            nc.vector.memset(ones_sb[:], 1.0)

            # ---------------- Phase 1: q/k/v projections + RoPE ----------
            with (
                tc.tile_pool(name="wqk", bufs=1) as wqk_pool,
                tc.tile_pool(name="xb", bufs=2) as x_pool,
                tc.tile_pool(name="wvt", bufs=3) as wv_pool,
                tc.tile_pool(name="trig", bufs=2) as trig_pool,
                tc.tile_pool(name="rope", bufs=3) as rope_pool,
                tc.tile_pool(name="qko", bufs=4) as qko_pool,
                tc.tile_pool(name="vo", bufs=3) as vo_pool,
                tc.tile_pool(name="psv", bufs=1, space="PSUM") as ps_v,
                tc.tile_pool(name="psqk", bufs=2, space="PSUM") as ps_qk,
            ):
                wq_sb = wqk_pool.tile([P, NKT * DPC], BF16, tag="wq")
                wk_sb = wqk_pool.tile([P, NKT * DPC], BF16, tag="wk")

                xtiles = {}

                def load_x(mc):
                    ms = mc * MC
                    xblk = x_pool.tile([P, NKT * MC], BF16, tag="xblk",
                                       name=f"xblk{mc}")
                    if mc == 0:
                        # land the k=0 slice first so the opening matmul
                        # doesn't wait on the full 2 MB block
                        nc.sync.dma_start(
                            xblk[:, 0:MC], xT_d[0:P, ms:ms + MC])
                        nc.sync.dma_start(
                            xblk[:, MC:].rearrange("p (k m) -> p k m",
                                                   k=NKT - 1),
                            xT_d[P:, ms:ms + MC].rearrange(
                                "(k p) m -> p k m", p=P),
                        )
                    else:
                        nc.sync.dma_start(
                            xblk[:].rearrange("p (k m) -> p k m", k=NKT),
                            xT_d[:, ms:ms + MC].rearrange(
                                "(k p) m -> p k m", p=P),
                        )
                    xtiles[mc] = xblk

                load_x(0)
                for mc in range(NMC):
                    ms = mc * MC
                    xblk = xtiles.pop(mc)
                    # --- v = x @ wv, row-major [rows, 512] ---
                    psv_t = [ps_v.tile([P, DPC], F32, tag=f"v{jj}",
                                       name=f"psv{jj}")
                             for jj in range(MC // P)]
                    for k in range(NKT):
                        wvt = wv_pool.tile([P, DPC], BF16)
                        nc.sync.dma_start(
                            wvt[:], wqkv_d[k * P:(k + 1) * P, 2 * DPC:3 * DPC])
                        for jj in range(MC // P):
                            nc.tensor.matmul(
                                psv_t[jj][:],
                                xblk[:, k * MC + jj * P: k * MC + (jj + 1) * P],
                                wvt[:],
                                start=(k == 0), stop=(k == NKT - 1),
                            )
                    if mc == 0:
                        # issued after the first v-projection work so the
                        # 8 MB q/k weight loads don't delay the first matmul
                        nc.sync.dma_start(
                            wq_sb[:].rearrange("p (k n) -> p k n", k=NKT),
                            wqkv_d[:, 0:DPC]
                                .rearrange("(k p) n -> p k n", p=P),
                        )
                        nc.sync.dma_start(
                            wk_sb[:].rearrange("p (k n) -> p k n", k=NKT),
                            wqkv_d[:, DPC:2 * DPC]
                                .rearrange("(k p) n -> p k n", p=P),
                        )
                    if mc + 1 < NMC:
                        # prefetch the next row chunk after this chunk's
                        # v-section DMAs so it doesn't delay the wv stream,
                        # but ahead of the stores below so the sync stream
                        # never head-of-line blocks the load
                        load_x(mc + 1)
                    for jj in range(MC // P):
                        vout = vo_pool.tile([P, DPC], BF16)
                        nc.vector.tensor_copy(vout[:], psv_t[jj][:])
                        r0 = ms + jj * P
                        nc.sync.dma_start(v_d[r0:r0 + P, :], vout[:])

                    # --- qT / kT with fused RoPE ---
                    cq = trig_pool.tile([P, MC], F32, tag="cq")
                    sq = trig_pool.tile([P, MC], F32, tag="sq")
                    ck = trig_pool.tile([P, MC], F32, tag="ck")
                    sk = trig_pool.tile([P, MC], F32, tag="sk")
                    nc.sync.dma_start(cq[:], cosq_d[:, ms:ms + MC])
                    nc.sync.dma_start(sq[:], sinq_d[:, ms:ms + MC])
                    nc.sync.dma_start(ck[:], cosk_d[:, ms:ms + MC])
                    nc.sync.dma_start(sk[:], sink_d[:, ms:ms + MC])

                    for w_sb, cos_t, sin_t, dest in (
                        (wq_sb, cq, sq, qT_d),
                        (wk_sb, ck, sk, kT_d),
                    ):
                        for nt in range(DPC // P):
                            psq = ps_qk.tile([P, MC], F32)
                            for k in range(NKT):
                                nc.tensor.matmul(
                                    psq[:],
                                    w_sb[:, k * DPC + nt * P: k * DPC + (nt + 1) * P],
                                    xblk[:, k * MC:(k + 1) * MC],
                                    start=(k == 0), stop=(k == NKT - 1),
                                )
                            cp = rope_pool.tile([P, MC], F32, tag="cp")
                            nc.scalar.activation(cp[:], psq[:], CopyF)
                            rot = rope_pool.tile([P, MC], F32, tag="rot")
                            nc.sync.dma_start(rot[0:64, :], cp[64:128, :])
                            nc.sync.dma_start(rot[64:128, :], cp[0:64, :])
                            tmp = rope_pool.tile([P, MC], F32, tag="tmp")
                            nc.vector.tensor_mul(tmp[:], psq[:], cos_t[:])
                            nc.vector.tensor_mul(rot[:], rot[:], sin_t[:])
                            ob = qko_pool.tile([P, MC], BF16)
                            nc.vector.tensor_add(ob[:], tmp[:], rot[:])
                            nc.sync.dma_start(
                                dest[nt * P:(nt + 1) * P, ms:ms + MC], ob[:])

            # ---------------- Phase 2: causal attention ------------------
            # wo_pool wraps phases 2+3 so the 8 MB o-projection weight load
            # can stream during attention instead of stalling phase 3.
            with (
                tc.tile_pool(name="wo3", bufs=1) as wo_pool,
                tc.tile_pool(name="mask2", bufs=1) as mask_pool,
                tc.tile_pool(name="ost", bufs=3) as ost_pool,
                tc.tile_pool(name="qk2", bufs=2) as qk2_pool,
                tc.tile_pool(name="v2", bufs=2) as v2_pool,
                tc.tile_pool(name="expb", bufs=6) as exp_pool,
                tc.tile_pool(name="norm", bufs=3) as norm_pool,
                tc.tile_pool(name="pss", bufs=3, space="PSUM") as ps_s,
                tc.tile_pool(name="pso", bufs=2, space="PSUM") as ps_o,
                tc.tile_pool(name="psc", bufs=2, space="PSUM") as ps_c,
                tc.tile_pool(name="ot3", bufs=2) as ot3_pool,
                tc.tile_pool(name="ev", bufs=4) as ev_pool,
            ):
                wo_sb = wo_pool.tile([P, HPC * HS], BF16, tag="wo")
                mask_sb = mask_pool.tile([P, 4 * MC], BF16, tag="mask")
                nc.sync.dma_start(mask_sb[:], mask_d[:])

                pairs = [(h, b) for h in range(HPC) for b in range(BS)]
                ptiles = {}

                def load_pair(idx):
                    h, b = pairs[idx]
                    c0 = b * SL
                    qt = qk2_pool.tile([P, SL], BF16, tag="q",
                                       name=f"q{idx}")
                    kt = qk2_pool.tile([P, SL], BF16, tag="k",
                                       name=f"k{idx}")
                    nc.sync.dma_start(
                        qt[:], qT_d[h * P:(h + 1) * P, c0:c0 + SL])
                    nc.sync.dma_start(
                        kt[:], kT_d[h * P:(h + 1) * P, c0:c0 + SL])
                    vt = v2_pool.tile([P, NJT * HD], BF16, tag="vt",
                                      name=f"vt{idx}")
                    nc.sync.dma_start(
                        vt[:].rearrange("p (j d) -> p j d", j=NJT),
                        v_d[c0:c0 + SL, h * HD:(h + 1) * HD]
                            .rearrange("(j p) d -> p j d", p=P),
                    )
                    ptiles[idx] = (qt, kt, vt)

                load_pair(0)
                for idx in range(len(pairs)):
                    if idx + 1 < len(pairs):
                        load_pair(idx + 1)
                    if idx == 0:
                        # behind the first two pairs' tiles, ahead of
                        # everything phase 3 needs
                        nc.sync.dma_start(
                            wo_sb[:].rearrange("p (a c) -> p a c", a=HPC),
                            wo_d.rearrange("(a p) c -> p a c", p=P),
                        )
                    h, b = pairs[idx]
                    c0 = b * SL
                    qt, kt, vt = ptiles.pop(idx)
                    if True:
                        for ic in range(NIC):
                            njt = 4 * (ic + 1)
                            ps_out = ps_o.tile([P, MC], F32)
                            ps_sum = ps_c.tile([P, MC], F32)
                            for jt in range(njt):
                                ps_sc = ps_s.tile([P, MC], F32)
                                nc.tensor.matmul(
                                    ps_sc[:],
                                    kt[:, jt * P:(jt + 1) * P],
                                    qt[:, ic * MC:(ic + 1) * MC],
                                    start=True, stop=True,
                                )
                                et = exp_pool.tile([P, MC], BF16)
                                nc.scalar.activation(et[:], ps_sc[:], ExpF)
                                if jt >= 4 * ic:
                                    t = jt - 4 * ic
                                    nc.vector.tensor_mul(
                                        et[:], et[:],
                                        mask_sb[:, t * MC:(t + 1) * MC])
                                nc.tensor.matmul(
                                    ps_out[:],
                                    vt[:, jt * HD:(jt + 1) * HD],
                                    et[:],
                                    start=(jt == 0), stop=(jt == njt - 1),
                                )
                                nc.tensor.matmul(
                                    ps_sum[:],
                                    ones_sb[:],
                                    et[:],
                                    start=(jt == 0), stop=(jt == njt - 1),
                                )
                            bcast = norm_pool.tile([P, MC], F32, tag="bcast")
                            nc.vector.reciprocal(bcast[:], ps_sum[:])
                            ost = ost_pool.tile([P, MC], BF16)
                            nc.vector.tensor_mul(
                                ost[:], ps_out[:], bcast[:])
                            nc.sync.dma_start(
                                oT_d[h * P:(h + 1) * P,
                                     c0 + ic * MC:c0 + (ic + 1) * MC],
                                ost[:])

                # ---------------- Phase 3: partial o-projection ----------
                # same pool scope as phase 2: wo_sb is already streaming in,
                # and ot loads are prefetched one row-chunk ahead so they
                # never queue behind this chunk's output stores.
                otiles = {}

                def load_ot(ic2):
                    ots = [ot3_pool.tile([P, MC], BF16, tag=f"ot{h}",
                                         name=f"ot{h}_{ic2}")
                           for h in range(HPC)]
                    for h in range(HPC):
                        nc.sync.dma_start(
                            ots[h][:],
                            oT_d[h * P:(h + 1) * P,
                                 ic2 * MC:(ic2 + 1) * MC])
                    otiles[ic2] = ots

                load_ot(0)
                for ic2 in range(ROWS // MC):
                    if ic2 + 1 < ROWS // MC:
                        load_ot(ic2 + 1)
                    ot_ic = otiles.pop(ic2)
                    for ct in range(HS // P):
                        # reuse phase 2's (now idle) PSUM banks: 4-way ILP
                        # without exceeding the 8-bank budget
                        psp = (ps_o if ct % 2 == 0 else ps_c).tile(
                            [P, MC], F32,
                            tag="ps_out" if ct % 2 == 0 else "ps_sum")
                        for h in range(HPC):
                            nc.tensor.matmul(
                                psp[:],
                                wo_sb[:, h * HS + ct * P: h * HS + (ct + 1) * P],
                                ot_ic[h][:],
                                start=(h == 0), stop=(h == HPC - 1),
                            )
                        ev = ev_pool.tile([P, MC], BF16)
                        # pin to DVE: ACT copies (~3.5us) can't keep pace
                        # with the PE's 0.85us-per-chunk rate here
                        nc.vector.tensor_copy(ev[:], psp[:])
                        nc.sync.dma_start(
                            out_d[ct * P:(ct + 1) * P,
                                  ic2 * MC:(ic2 + 1) * MC], ev[:])

    nc.compile()
    return nc


def _host_inputs(hidden_states, wq, wk, wv, wo):
    """Per-core input dicts: xT (replicated), packed wqkv slice, wo slice."""
    x = np.asarray(hidden_states, dtype=np.float32).reshape(ROWS, HS)
    xT = np.ascontiguousarray(x.T).astype(BFNP)

    wq = np.asarray(wq, np.float32)
    wk = np.asarray(wk, np.float32)
    wv = np.asarray(wv, np.float32)
    wo = np.asarray(wo, np.float32)

    in_maps = []
    for c in range(N_CORES):
        s = slice(c * DPC, (c + 1) * DPC)
        wqkv = np.concatenate([wq[:, s], wk[:, s], wv[:, s]], axis=1)
        in_maps.append({
            "xT": xT,
            "wqkv": np.ascontiguousarray(wqkv).astype(BFNP),
            "wo": np.ascontiguousarray(wo[s, :]).astype(BFNP),
        })
    return in_maps


class Runner:
    """Compile the program once into a sharded PJRT executable with the
    bass effect suppressed (C++ fast-path dispatch). Inputs must be
    device-resident with the mesh sharding; use stage() for that."""

    def __init__(self, nc):
        import jax
        import concourse.mybir as _mybir
        from concourse import bass2jax
        from jax.experimental.shard_map import shard_map
        from jax.sharding import Mesh, PartitionSpec, NamedSharding

        bass2jax.install_neuronx_cc_hook()
        self.jax = jax
        partition_name = (
            nc.partition_id_tensor.name if nc.partition_id_tensor else None)
        in_names, in_avals, out_names, out_avals = [], [], [], []
        for alloc in nc.m.functions[0].allocations:
            if not isinstance(alloc, _mybir.MemoryLocationSet):
                continue
            name = alloc.memorylocations[0].name
            if alloc.kind == "ExternalInput":
                if name != partition_name:
                    in_names.append(name)
                    in_avals.append((tuple(alloc.tensor_shape),
                                     _mybir.dt.np(alloc.dtype)))
            elif alloc.kind == "ExternalOutput":
                shape = tuple(alloc.tensor_shape)
                dtype = _mybir.dt.np(alloc.dtype)
                out_names.append(name)
                out_avals.append(jax.core.ShapedArray(shape, dtype))
        self.in_names, self.out_names = in_names, out_names
        self.out_avals = out_avals
        all_names = list(in_names)
        if partition_name is not None:
            all_names = all_names + [partition_name]

        def _body(*args):
            operands = list(args)
            if partition_name is not None:
                operands.append(bass2jax.partition_id_tensor())
            outs = bass2jax._bass_exec_p.bind(
                *operands,
                out_avals=tuple(out_avals),
                in_names=tuple(all_names),
                out_names=tuple(out_names),
                lowering_input_output_aliases=(),
                sim_require_finite=True,
                sim_require_nnan=True,
                nc=nc,
            )
            return tuple(outs)

        devices = jax.devices()[:N_CORES]
        self.mesh = Mesh(np.asarray(devices), ("core",))
        self.sharding = NamedSharding(self.mesh, PartitionSpec("core"))
        wrapped = shard_map(
            _body, mesh=self.mesh,
            in_specs=(PartitionSpec("core"),) * len(in_names),
            out_specs=(PartitionSpec("core"),) * len(out_names),
            check_rep=False,
        )
        abstract = [
            jax.ShapeDtypeStruct((N_CORES * shape[0],) + shape[1:], dtype,
                                 sharding=self.sharding)
            for shape, dtype in in_avals
        ]
        self.fn = bass2jax.fast_dispatch_compile(
            lambda: jax.jit(wrapped, keep_unused=True)
            .lower(*abstract).compile())

    def concat_inputs(self, in_maps):
        return [
            np.concatenate([np.asarray(m[name]) for m in in_maps], axis=0)
            for name in self.in_names
        ]

    def stage(self, in_maps):
        """Concatenate per-core inputs and place them on the mesh."""
        args = self.concat_inputs(in_maps)
        dev_args = [self.jax.device_put(a, self.sharding) for a in args]
        self.jax.block_until_ready(dev_args)
        return dev_args

    def run(self, in_maps):
        out_arrs = self.fn(*self.stage(in_maps))
        return [
            {
                name: np.asarray(out_arrs[i]).reshape(
                    N_CORES, *self.out_avals[i].shape)[c]
                for i, name in enumerate(self.out_names)
            }
            for c in range(N_CORES)
        ]


_RUNNER = None


def get_runner():
    global _RUNNER
    if _RUNNER is None:
        _RUNNER = Runner(build_program())
    return _RUNNER


def kernel(hidden_states, wq, wk, wv, wo):
    runner = get_runner()
    in_maps = _host_inputs(hidden_states, wq, wk, wv, wo)
    results = runner.run(in_maps)
    total = results[0]["out"].astype(np.float64)
    for c in range(1, N_CORES):
        total += results[c]["out"].astype(np.float64)
    return np.ascontiguousarray(
        total.T.reshape(BS, SL, HS)).astype(np.float32)
